# revision 20
# baseline (speedup 1.0000x reference)
"""Multi-head causal attention (B=2, T=2048, C=1024, H=16) on 8 Trainium2
NeuronCores, tensor-parallel over heads (2 heads per core).

v2 — restructured from the v1 flash-style kernel for PE saturation:
  - Phase A first: QKV projection for whole batches up front (PE runs long
    uninterrupted matmul streams, reaching/keeping the 2.4 GHz p-state),
    batched x loads (1 DMA per chunk), V transposed into [token, dim]
    layout via DMA-transpose (xbar) instead of PE transposes + DVE copies.
  - Phase B per (batch, q-chunk): scores for BOTH heads go into one
    2-bank PSUM tile [128, 1024]; ONE exp activation covers both heads.
    Causal masking is multiplicative {0,1} bf16 AFTER the exp (DVE, 2x
    mode) so the scores->exp chain has no DVE hop; only diagonal k-tiles
    are masked. AV keeps the ones-column trick (row 64 = softmax denom).
  - Normalize: denominators broadcast via stacked f32r outer-product
    matmuls (h1 lands at PSUM partitions 64-127 via tile_position), one
    fast reciprocal, two DVE muls write a SINGLE stacked [128, 512] ot
    tile (h1 write shifts partitions 0-63 -> 64-127).
  - Out-projection contracts the full 128 partitions in one matmul per
    m-tile (half the v1 matmuls), drains split Act/DVE, one y DMA per
    chunk, yt output in bf16 (halves write traffic).
  - Software pipelining: each chunk's normalize+out-projection is emitted
    a few k-tiles INTO the next chunk's score loop (so the PE never waits
    on the DVE normalize chain), and batch-1's QKV chunks are interleaved
    between batch-0 attention chunks as PE filler.

Matmuls in bf16 (fp32 PSUM); softmax normalization in fp32/f32r.
"""

import os
import sys

for _p in ("/opt/trn_rl_repo", "/root/.axon_site/_ro/trn_rl_repo"):
    if os.path.isdir(_p) and _p not in sys.path:
        sys.path.insert(0, _p)

import ml_dtypes
import numpy as np

import concourse.bacc as bacc
import concourse.bass as bass
import concourse.mybir as mybir
import concourse.tile as tile
from concourse.bass_utils import run_bass_kernel_spmd
from concourse.masks import make_identity

B, T, C, H, D = 2, 2048, 1024, 16, 64
NCORES = 8
BT = B * T                      # 4096 flattened tokens
TC = 512                        # token chunk (matmul free dim)
NTC = BT // TC                  # 8 token chunks
FP = mybir.dt.float32
FPR = mybir.dt.float32r
BF = mybir.dt.bfloat16
ACT = mybir.ActivationFunctionType
AV_DELAY = 3                    # k-tiles the AV matmul trails the scores

LAST_RESULTS = None             # stashed BassKernelResults for test harness


def build_nc():
    nc = bacc.Bacc(None, target_bir_lowering=False, debug=False)

    xt = nc.declare_dram_parameter("xt", [C, BT], BF, isOutput=False)
    wc = nc.declare_dram_parameter("wc", [C, 384], BF, isOutput=False)
    wout = nc.declare_dram_parameter("wout", [128, C], BF, isOutput=False)
    bqkv = nc.declare_dram_parameter("bqkv", [128, 3], FP, isOutput=False)
    masks = nc.declare_dram_parameter("masks", [512, 1024], BF, isOutput=False)
    ones = nc.declare_dram_parameter("ones", [128, 64], BF, isOutput=False)
    # [33, 128] selector: row 0 -> output partitions 0-63 (head 0), row 32
    # -> partitions 64-127 (head 1), rows 1-31 zero (engine partition bases
    # must be multiples of 32). One f32r matmul broadcasts both heads'
    # softmax denominators.
    onesr = nc.declare_dram_parameter("onesr", [33, 128], FP, isOutput=False)
    yt = nc.declare_dram_parameter("yt", [C, BT], BF, isOutput=True)

    with tile.TileContext(nc) as tc:
        with (
            tc.tile_pool(name="const", bufs=1) as cpool,
            tc.tile_pool(name="big", bufs=1) as bigpool,
            tc.tile_pool(name="sb", bufs=2) as sbpool,
            tc.tile_pool(name="ps", bufs=2, space="PSUM") as pspool,
        ):
            # ---- constants ----
            wc_sb = cpool.tile([128, 8 * 384], BF)      # [cin, k*384 + g*128 + col]
            nc.sync.dma_start(
                out=wc_sb[:].rearrange("b (a c) -> b a c", a=8),
                in_=wc.rearrange("(a b) c -> b a c", a=8),
            )
            wout_sb = cpool.tile([128, C], BF)
            nc.sync.dma_start(out=wout_sb[:], in_=wout[:, :])
            bq_sb = cpool.tile([128, 3], FP)
            nc.sync.dma_start(out=bq_sb[:], in_=bqkv[:, :])
            # multiplicative causal masks for the 4 diagonal k-tiles,
            # duplicated for both heads: [128, v*1024 + h*512 + q]
            masks_sb = cpool.tile([128, 4 * 1024], BF)
            nc.sync.dma_start(
                out=masks_sb[:].rearrange("b (a c) -> b a c", a=4),
                in_=masks.rearrange("(a b) c -> b a c", a=4),
            )
            onesr_sb = cpool.tile([33, 128], FP)
            nc.sync.dma_start(out=onesr_sb[:], in_=onesr[:, :])
            # two static rcsum staging tiles (alternated per chunk), zeroed
            # once so selector rows 1-31 always multiply zeros
            rc_tiles = []
            for i in range(2):
                rct = cpool.tile([33, TC], FP, name=f"rcst{i}")
                nc.vector.memset(rct[:], 0.0)
                rc_tiles.append(rct)
            ident = cpool.tile([128, 128], BF)
            make_identity(nc, ident)

            # ---- persistent intermediates ----
            QT = bigpool.tile([128, BT], BF)
            KT = bigpool.tile([128, BT], BF)
            # V in [token, dim] layout, 130 cols per 128-token block:
            # [V_h0 (64) | ones | V_h1 (64) | ones]
            vaug = bigpool.tile([128, 32 * 130], BF)
            nc.sync.dma_start(
                out=vaug[:].rearrange("p (j a c) -> p j a c", a=2, c=65)[
                    :, :, :, 64:65],
                in_=ones.rearrange("p (j a c) -> p j a c", a=2, c=1)[:, 0:32],
            )

            def phase_a(tcx):
                """QKV projection + V transpose for one 512-token chunk."""
                t0 = tcx * TC
                xtile = sbpool.tile([128, 8 * TC], BF, tag="xt", bufs=3)
                nc.sync.dma_start(
                    out=xtile[:].rearrange("p (a c) -> p a c", a=8),
                    in_=xt.rearrange("(a b) c -> b a c", a=8)[:, :, t0:t0 + TC],
                )
                vt = None
                for g in range(3):
                    # share the "s" PSUM ring with phase B (8-bank budget);
                    # only the first 512 columns (1 bank) are used here
                    ps = pspool.tile([128, 2 * TC], FP, tag="s", bufs=2)
                    for k in range(8):
                        nc.tensor.matmul(
                            ps[:, 0:TC],
                            wc_sb[:, k * 384 + g * 128:k * 384 + (g + 1) * 128],
                            xtile[:, k * TC:(k + 1) * TC],
                            start=(k == 0),
                            stop=(k == 7),
                        )
                    if g < 2:
                        dst = (QT, KT)[g]
                        nc.scalar.activation(
                            dst[:, t0:t0 + TC], ps[:, 0:TC], ACT.Identity,
                            bias=bq_sb[:, g:g + 1],
                        )
                    else:
                        vt = sbpool.tile([128, TC], BF, tag="vt", bufs=2)
                        nc.scalar.activation(
                            vt[:], ps[:, 0:TC], ACT.Identity,
                            bias=bq_sb[:, 2:3],
                        )
                # V transpose into vaug: PE transpose per 128-token block
                # (bf16, into a y-ring bank bitcast to bf16), then one
                # strided DVE copy splits the two heads around the ones col
                for j in range(4):
                    jj = tcx * 4 + j
                    tp = pspool.tile([128, TC], FP, tag="y", bufs=2,
                                     name="tp")
                    tpb = tp[:].bitcast(BF)
                    nc.tensor.transpose(
                        tpb[:, 0:128], vt[:, j * 128:(j + 1) * 128], ident[:]
                    )
                    nc.vector.tensor_copy(
                        vaug[:].rearrange("p (j a c) -> p j a c", a=2, c=65)[
                            :, jj, :, 0:64],
                        tpb[:, 0:128].rearrange("p (a c) -> p a c", c=64),
                    )

            def attn(tcx, at_kt2=None):
                """Scores/exp/AV for one (batch, q-chunk). Returns a
                finish() closure (normalize + out-projection) that the
                schedule emits later, pipelined behind newer PE work.
                at_kt2: callback emitted right after k-tile 2's exp (before
                the first AV) — used to place the PREVIOUS chunk's finish
                so its PSUM rings free up in time without deadlocking."""
                b, qc = divmod(tcx, 4)
                t0 = tcx * TC
                n_kt = 4 * (qc + 1)

                otp = pspool.tile([65, 2 * TC], FP, tag="av", bufs=1,
                                  name="otp")
                pts = {}

                def emit_av(j):
                    kg = b * 16 + j
                    pt = pts.pop(j)
                    for h in range(2):
                        nc.tensor.matmul(
                            otp[:, h * TC:(h + 1) * TC],
                            vaug[:, kg * 130 + h * 65:kg * 130 + h * 65 + 65],
                            pt[:, h * TC:(h + 1) * TC],
                            start=(j == 0), stop=(j == n_kt - 1),
                            skip_group_check=True,
                        )

                for kt in range(n_kt):
                    kg = b * 16 + kt
                    sp = pspool.tile([128, 2 * TC], FP, tag="s", bufs=2)
                    for h in range(2):
                        nc.tensor.matmul(
                            sp[:, h * TC:(h + 1) * TC],
                            KT[h * 64:(h + 1) * 64, kg * 128:(kg + 1) * 128],
                            QT[h * 64:(h + 1) * 64, t0:t0 + TC],
                            start=True, stop=True,
                        )
                    pt = sbpool.tile([128, 2 * TC], BF, tag="pt",
                                     bufs=AV_DELAY + 4)
                    nc.scalar.activation(pt[:], sp[:], ACT.Exp, scale=0.125)
                    if kt >= 4 * qc:
                        v = kt - 4 * qc
                        nc.vector.tensor_mul(
                            pt[:], pt[:],
                            masks_sb[:, v * 1024:(v + 1) * 1024],
                        )
                    pts[kt] = pt
                    if kt == 2 and at_kt2 is not None:
                        at_kt2()
                    if kt >= AV_DELAY:
                        emit_av(kt - AV_DELAY)
                for j in range(max(n_kt - AV_DELAY, 0), n_kt):
                    emit_av(j)

                def finish():
                    # ---- normalize: stacked 1/rowsum broadcast ----
                    rcsum = rc_tiles[tcx % 2]
                    nc.vector.tensor_copy(rcsum[0:1, :], otp[64:65, 0:TC])
                    nc.vector.tensor_copy(rcsum[32:33, :],
                                          otp[64:65, TC:2 * TC])
                    bc = pspool.tile([128, TC], FP, tag="y", bufs=2,
                                     name="bc")
                    nc.tensor.matmul(bc[:], onesr_sb[:], rcsum[:],
                                     start=True, stop=True)
                    bcs = sbpool.tile([128, TC], FP, tag="bcs", bufs=2,
                                      name="bcs")
                    nc.vector.reciprocal_approx_fast(out=bcs[:], in_=bc[:])
                    ots = sbpool.tile([128, TC], BF, tag="ot", bufs=2,
                                      name="ot")
                    nc.vector.tensor_mul(ots[0:64, :], otp[0:64, 0:TC],
                                         bcs[0:64, :])
                    nc.vector.tensor_mul(ots[64:128, :], otp[0:64, TC:2 * TC],
                                         bcs[64:128, :])

                    # ---- out-projection: full 128-contract per m-tile ----
                    ys = sbpool.tile([128, 8 * TC], BF, tag="ys", bufs=2)
                    for m in range(8):
                        yp = pspool.tile([128, TC], FP, tag="y", bufs=2)
                        nc.tensor.matmul(
                            yp[:], wout_sb[:, m * 128:(m + 1) * 128], ots[:],
                            start=True, stop=True,
                        )
                        if m % 2 == 0:
                            nc.scalar.copy(ys[:, m * TC:(m + 1) * TC], yp[:])
                        else:
                            nc.vector.tensor_copy(
                                ys[:, m * TC:(m + 1) * TC], yp[:])
                    nc.sync.dma_start(
                        out=yt.rearrange("(a b) c -> b a c", a=8)[
                            :, :, t0:t0 + TC],
                        in_=ys[:].rearrange("p (a c) -> p a c", a=8),
                    )

                return finish

            # ---- schedule ----
            # batch-0 QKV; then attention chunks with batch-1 QKV chunks
            # interleaved as PE filler. Each chunk's finish (normalize +
            # out-projection) is emitted at k-tile 2 of the NEXT chunk,
            # before that chunk's first AV (av ring bufs=1 would deadlock
            # otherwise) and behind enough PE work to hide the DVE chain.
            for tcx in range(4):
                phase_a(tcx)
            pending = None
            for qc in range(4):
                pending = attn(qc, at_kt2=pending)
                phase_a(4 + qc)
            for qc in range(4):
                pending = attn(4 + qc, at_kt2=pending)
            pending()

    nc.compile()
    return nc


def make_in_maps(x, w_qkv, b_qkv):
    x = np.ascontiguousarray(np.asarray(x, np.float32).reshape(BT, C))
    xT = np.ascontiguousarray(x.T).astype(ml_dtypes.bfloat16)
    w_qkv = np.asarray(w_qkv, np.float32)
    b_qkv = np.asarray(b_qkv, np.float32)

    # multiplicative causal mask for diagonal k-tiles, duplicated per head
    mask = np.empty((512, 1024), np.float32)
    for v in range(4):
        kk = np.arange(128)[:, None] + 128 * v
        qq = np.arange(512)[None, :]
        m = (kk <= qq).astype(np.float32)
        mask[v * 128:(v + 1) * 128, 0:512] = m
        mask[v * 128:(v + 1) * 128, 512:1024] = m
    mask = mask.astype(ml_dtypes.bfloat16)

    onesr = np.zeros((33, 128), np.float32)
    onesr[0, 0:64] = 1.0
    onesr[32, 64:128] = 1.0

    in_maps = []
    for c in range(NCORES):
        sl = slice(c * 128, (c + 1) * 128)
        wcs = np.concatenate(
            [w_qkv[:, sl], w_qkv[:, 1024:][:, sl], w_qkv[:, 2048:][:, sl]], axis=1
        )
        bq = np.stack(
            [b_qkv[sl], b_qkv[1024:][sl], b_qkv[2048:][sl]], axis=1
        )
        in_maps.append({
            "xt": xT,
            "wc": np.ascontiguousarray(wcs).astype(ml_dtypes.bfloat16),
            "wout": None,  # filled by caller (needs w_out)
            "bqkv": np.ascontiguousarray(bq),
            "masks": mask,
            "ones": np.ones((128, 64), ml_dtypes.bfloat16),
            "onesr": onesr,
        })
    return in_maps


_NC_CACHE = None


def kernel(x, w_qkv, b_qkv, w_out, b_out):
    global _NC_CACHE, LAST_RESULTS
    if _NC_CACHE is None:
        _NC_CACHE = build_nc()
    nc = _NC_CACHE

    w_out = np.asarray(w_out, np.float32)
    in_maps = make_in_maps(x, w_qkv, b_qkv)
    for c in range(NCORES):
        in_maps[c]["wout"] = np.ascontiguousarray(
            w_out[c * 128:(c + 1) * 128, :]).astype(ml_dtypes.bfloat16)

    res = run_bass_kernel_spmd(
        nc, in_maps, list(range(NCORES)),
        trace=bool(os.environ.get("BASS_TRACE")),
    )
    LAST_RESULTS = res

    acc = np.zeros((C, BT), np.float32)
    for out_map in res.results:
        acc += out_map["yt"].astype(np.float32)
    y = acc.T + np.asarray(b_out, np.float32)[None, :]
    return y.reshape(B, T, C)
